# revision 1
# baseline (speedup 1.0000x reference)
"""MultiHeadAttention Trainium2 kernel (8 NeuronCores, Bass/Tile).

Problem: B=2, S=2048, D=1024, H=16, DK=64 fp32 MHA (torch-Linear style
projections, softmax attention, output projection).

Sharding: core c = (batch b = c//4, head-group g = c%4); each core handles
4 heads of one batch, entirely in a transposed layout (features on
partitions, sequence on the free axis):
  qhT/khT  = (W_g x^T + b)       [2 pairs x 128, 2048]
  vh       = x_v Wv_g^T          [2048, 4x65] (ones col -> row sums)
  scoresT  = khT^T qhT           per (pair, ktile, qtile) -> PSUM
  expT     = exp(scoresT/8 - 2)  ACT (bias -2 for fp16 headroom)
  rawT     = vh_aug^T expT       PV matmul; row 64 = softmax denominator
  outT     = rawT[0:64] * (1/rawT[64])
  partialT = woT^T outT          [1024, 2048] fp32 -> DRAM
Host: out[b] = sum_g partialT(b,g)^T + (Wo bv + bo).

PE is exact on fp16/bf16 operands (fp32 accumulate); per-stage operand
dtypes are configurable below. Softmax denominators come free via the
ones column (attention rows sum to 1, which also lets Wo@bv fold into a
host-side constant). No collectives; host sums 4 partials per batch.
"""

import numpy as np

B, S, D, H = 2, 2048, 1024, 16
DK = D // H          # 64
N_CORES = 8
HG = H // 4          # 4 head-groups
HL = 4               # heads per core
FEAT = HL * DK       # 256 per-core features
NQT = S // 512       # 4 query tiles
NKT = S // 128       # 16 key tiles
NDT = D // 128       # 8 contraction tiles (d-model)

# per-stage matmul operand dtypes ("fp16" | "bf16")
DT_QK = "fp16"   # x_q/x_k, Wq/Wk, qhT/khT (score operands)
DT_V = "fp16"    # x_v, Wv
DT_PV = "bf16"   # vh_aug, expT
DT_O = "fp16"    # Wo, outT (feeds final output directly)

_cache = {}


def _np_dt(name):
    if name == "fp16":
        return np.float16
    import ml_dtypes
    return ml_dtypes.bfloat16


def _build():
    import concourse.mybir as mybir
    import concourse.tile as tile
    from concourse import bacc

    fp32 = mybir.dt.float32
    dt_qk = getattr(mybir.dt, "float16" if DT_QK == "fp16" else "bfloat16")
    dt_v = getattr(mybir.dt, "float16" if DT_V == "fp16" else "bfloat16")
    dt_pv = getattr(mybir.dt, "float16" if DT_PV == "fp16" else "bfloat16")
    dt_o = getattr(mybir.dt, "float16" if DT_O == "fp16" else "bfloat16")

    nc = bacc.Bacc("TRN2", target_bir_lowering=False, debug=False,
                   num_devices=N_CORES)

    xqT = nc.dram_tensor("xqT", [D, S], dt_qk, kind="ExternalInput").ap()
    xkT = nc.dram_tensor("xkT", [D, S], dt_qk, kind="ExternalInput").ap()
    xvT = nc.dram_tensor("xvT", [D, S], dt_v, kind="ExternalInput").ap()
    wqT = nc.dram_tensor("wqT", [D, FEAT], dt_qk, kind="ExternalInput").ap()
    wkT = nc.dram_tensor("wkT", [D, FEAT], dt_qk, kind="ExternalInput").ap()
    wvT = nc.dram_tensor("wvT", [D, FEAT], dt_v, kind="ExternalInput").ap()
    woT = nc.dram_tensor("woT", [FEAT, D], dt_o, kind="ExternalInput").ap()
    bq2 = nc.dram_tensor("bq2", [FEAT, 1], fp32, kind="ExternalInput").ap()
    bk2 = nc.dram_tensor("bk2", [FEAT, 1], fp32, kind="ExternalInput").ap()
    out_d = nc.dram_tensor("partialT", [D, S], fp32, kind="ExternalOutput").ap()

    xq_r = xqT.rearrange("(t p) s -> p t s", p=128)
    xk_r = xkT.rearrange("(t p) s -> p t s", p=128)
    xv_r = xvT.rearrange("(t p) s -> p t s", p=128)

    with tile.TileContext(nc) as tc:
        with (
            tc.tile_pool(name="xin", bufs=1) as xin,
            tc.tile_pool(name="win", bufs=1) as win,
            tc.tile_pool(name="proj", bufs=1) as proj,
            tc.tile_pool(name="pexp", bufs=17) as pexp,
            tc.tile_pool(name="pout", bufs=4) as pout,
            tc.tile_pool(name="pnrm", bufs=2) as pnrm,
            tc.tile_pool(name="pp", bufs=2, space="PSUM") as pp,
            tc.tile_pool(name="ps2", bufs=2, space="PSUM") as ps2,
            tc.tile_pool(name="pspv", bufs=2, space="PSUM") as pspv,
        ):
            # ---- load inputs: weights first (small, unblock compute),
            # then x per d-tile in consumption order (v, then q/k) ----
            wq3 = win.tile([128, NDT, FEAT], dt_qk, tag="wq")
            wk3 = win.tile([128, NDT, FEAT], dt_qk, tag="wk")
            wv3 = win.tile([128, NDT, FEAT], dt_v, tag="wv")
            wo3 = win.tile([128, 2, D], dt_o, tag="wo")
            bq3 = win.tile([128, 2, 1], fp32, tag="bq")
            bk3 = win.tile([128, 2, 1], fp32, tag="bk")
            nc.sync.dma_start(wk3[:], wkT.rearrange("(t p) f -> p t f", p=128))
            nc.sync.dma_start(wq3[:], wqT.rearrange("(t p) f -> p t f", p=128))
            nc.sync.dma_start(wv3[:], wvT.rearrange("(t p) f -> p t f", p=128))
            nc.sync.dma_start(wo3[:], woT.rearrange("(t p) j -> p t j", p=128))
            nc.sync.dma_start(bq3[:], bq2.rearrange("(t p) o -> p t o", p=128))
            nc.sync.dma_start(bk3[:], bk2.rearrange("(t p) o -> p t o", p=128))
            xq3 = xin.tile([128, NDT, S], dt_qk, tag="xq")
            xk3 = xin.tile([128, NDT, S], dt_qk, tag="xk")
            xv3 = xin.tile([128, NDT, S], dt_v, tag="xv")
            for t in range(NDT):
                nc.sync.dma_start(xk3[:, t, :], xk_r[:, t, :])
                nc.sync.dma_start(xq3[:, t, :], xq_r[:, t, :])
            for t in range(NDT):
                nc.sync.dma_start(xv3[:, t, :], xv_r[:, t, :])

            # ---- persistent intermediates ----
            qh3 = proj.tile([128, 2, S], dt_qk, tag="qh")   # pair-packed
            kh3 = proj.tile([128, 2, S], dt_qk, tag="kh")
            vha = proj.tile([128, NKT, HL, DK + 1], dt_pv, tag="vha")
            ot3 = proj.tile([128, 2, S], dt_o, tag="outT")

            nc.gpsimd.memset(vha[:, :, :, DK], 1.0)  # ones column
            # exp bias -2: headroom under fp16 max (cancels in division)
            ebias = win.tile([128, 1], fp32, tag="ebias")
            nc.gpsimd.memset(ebias[:], -2.0)

            # ---- projections ----
            # emission order: q/k for pair 0 first, then v, then q/k pair 1 —
            # pair-0 scores/exp become schedulable early, keeping ACT busy
            # while the remaining projections still occupy the PE.
            def qk_proj(m):
                for x3, w3, b3, dst in ((xk3, wk3, bk3, kh3),
                                        (xq3, wq3, bq3, qh3)):
                    for n in range(NQT):
                        ps = pp.tile([128, 512], fp32, tag="acc")
                        for kt in range(NDT):
                            nc.tensor.matmul(
                                ps[:],
                                w3[:, kt, m * 128:(m + 1) * 128],
                                x3[:, kt, n * 512:(n + 1) * 512],
                                start=(kt == 0), stop=(kt == NDT - 1))
                        nc.vector.tensor_scalar_add(
                            dst[:, m, n * 512:(n + 1) * 512], ps[:], b3[:, m, :])

            def v_proj():
                for st in range(NKT):
                    ps = pp.tile([128, 256], fp32, tag="acc")
                    for kt in range(NDT):
                        nc.tensor.matmul(
                            ps[:], xv3[:, kt, st * 128:(st + 1) * 128],
                            wv3[:, kt, :],
                            start=(kt == 0), stop=(kt == NDT - 1))
                    nc.vector.tensor_copy(vha[:, st, :, 0:DK], ps[:])

            # ---- attention (split so scores/exp of (0,0) can be
            # emitted before v-proj and qk_proj(1), starting ACT ~35us
            # earlier; PV readers are emitted only after v-proj writes) ----
            def attn_scores(qt, hp):
                e2s = []
                for kt in range(NKT):
                    s2 = ps2.tile([128, 1024], fp32, tag="s2")
                    nc.tensor.matmul(
                        s2[:, 0:512],
                        kh3[0:64, hp, kt * 128:(kt + 1) * 128],
                        qh3[0:64, hp, qt * 512:(qt + 1) * 512],
                        start=True, stop=True)
                    nc.tensor.matmul(
                        s2[:, 512:1024],
                        kh3[64:128, hp, kt * 128:(kt + 1) * 128],
                        qh3[64:128, hp, qt * 512:(qt + 1) * 512],
                        start=True, stop=True)
                    e2 = pexp.tile([128, 1024], dt_pv, tag="e2")
                    if DT_PV == "bf16":   # bf16 range: no overflow risk
                        nc.scalar.activation(
                            e2[:], s2[:],
                            mybir.ActivationFunctionType.Exp, scale=0.125)
                    else:
                        nc.scalar.activation(
                            e2[:], s2[:],
                            mybir.ActivationFunctionType.Exp,
                            scale=0.125, bias=ebias[:])
                    e2s.append(e2)
                return e2s

            def attn_pv(qt, hp, e2s):
                pva = pspv.tile([DK + 1, 512], fp32, tag="pv")
                pvb = pspv.tile([DK + 1, 512], fp32, tag="pv")
                for kt in range(NKT):
                    nc.tensor.matmul(
                        pva[:], vha[:, kt, 2 * hp, :], e2s[kt][:, 0:512],
                        start=(kt == 0), stop=(kt == NKT - 1))
                    nc.tensor.matmul(
                        pvb[:], vha[:, kt, 2 * hp + 1, :],
                        e2s[kt][:, 512:1024],
                        start=(kt == 0), stop=(kt == NKT - 1))
                for pv, half in ((pva, 0), (pvb, 1)):
                    # custom DVE ops must read SBUF, not PSUM
                    srow = pnrm.tile([1, 512], fp32, tag="srow")
                    nc.vector.tensor_copy(srow[:], pv[DK:DK + 1, :])
                    inv = pnrm.tile([1, 512], fp32, tag="inv")
                    nc.vector.reciprocal_approx_fast(inv[:], srow[:])
                    invb = pnrm.tile([64, 512], fp32, tag="invb")
                    nc.gpsimd.partition_broadcast(invb[:], inv[:])
                    nc.vector.tensor_tensor(
                        ot3[half * 64:(half + 1) * 64, hp,
                            qt * 512:(qt + 1) * 512],
                        pv[0:DK, :], invb[:], mybir.AluOpType.mult)

            def oproj(qt):
                for jt in range(NDT):
                    ps = pp.tile([128, 512], fp32, tag="acc")
                    for m in range(2):
                        nc.tensor.matmul(
                            ps[:], wo3[:, m, jt * 128:(jt + 1) * 128],
                            ot3[:, m, qt * 512:(qt + 1) * 512],
                            start=(m == 0), stop=(m == 1))
                    po = pout.tile([128, 512], fp32, tag="po")
                    nc.vector.tensor_copy(po[:], ps[:])
                    nc.sync.dma_start(
                        out_d[jt * 128:(jt + 1) * 128,
                              qt * 512:(qt + 1) * 512], po[:])

            qk_proj(0)
            e00 = attn_scores(0, 0)   # ACT starts here, during qk1/v-proj
            qk_proj(1)
            v_proj()
            attn_pv(0, 0, e00)
            e01 = attn_scores(0, 1)
            attn_pv(0, 1, e01)
            oproj(0)
            for qt in range(1, NQT):
                for hp in range(2):
                    e = attn_scores(qt, hp)
                    attn_pv(qt, hp, e)
                oproj(qt)

    nc.compile()
    return nc


def kernel(q, k, v, Wq, bq, Wk, bk, Wv, bv, Wo, bo, _trace=False):
    from concourse import bass_utils

    if "nc" not in _cache:
        _cache["nc"] = _build()
    nc = _cache["nc"]

    q = np.asarray(q, np.float32)
    k = np.asarray(k, np.float32)
    v = np.asarray(v, np.float32)
    Wq = np.asarray(Wq, np.float32)
    Wk = np.asarray(Wk, np.float32)
    Wv = np.asarray(Wv, np.float32)
    Wo = np.asarray(Wo, np.float32)
    bq = np.asarray(bq, np.float32)
    bk = np.asarray(bk, np.float32)
    bv = np.asarray(bv, np.float32)
    bo = np.asarray(bo, np.float32)

    d_qk, d_v, d_o = _np_dt(DT_QK), _np_dt(DT_V), _np_dt(DT_O)
    xT = {}
    for b in range(B):
        xT[("q", b)] = np.ascontiguousarray(q[b].T).astype(d_qk)
        xT[("k", b)] = np.ascontiguousarray(k[b].T).astype(d_qk)
        xT[("v", b)] = np.ascontiguousarray(v[b].T).astype(d_v)
    wT = {}
    for g in range(HG):
        sl = slice(g * FEAT, (g + 1) * FEAT)
        wT[("q", g)] = np.ascontiguousarray(Wq[sl, :].T).astype(d_qk)
        wT[("k", g)] = np.ascontiguousarray(Wk[sl, :].T).astype(d_qk)
        wT[("v", g)] = np.ascontiguousarray(Wv[sl, :].T).astype(d_v)
        wT[("o", g)] = np.ascontiguousarray(Wo[:, sl].T).astype(d_o)

    in_maps = []
    for c in range(N_CORES):
        b, g = divmod(c, HG)
        sl = slice(g * FEAT, (g + 1) * FEAT)
        in_maps.append({
            "xqT": xT[("q", b)], "xkT": xT[("k", b)], "xvT": xT[("v", b)],
            "wqT": wT[("q", g)], "wkT": wT[("k", g)], "wvT": wT[("v", g)],
            "woT": wT[("o", g)],
            "bq2": np.ascontiguousarray(bq[sl]).reshape(FEAT, 1),
            "bk2": np.ascontiguousarray(bk[sl]).reshape(FEAT, 1),
        })

    kwargs = {}
    if _trace:
        _install_profile_shim()
        kwargs = dict(trace=True, trace_cores=list(range(N_CORES)))
    res = bass_utils.run_bass_kernel_spmd(
        nc, in_maps, core_ids=list(range(N_CORES)), **kwargs)
    _cache["last_results"] = res

    final_bias = (Wo @ bv + bo).astype(np.float32)  # attn rows sum to 1
    out = np.empty((B, S, D), np.float32)
    for b in range(B):
        acc = res.results[b * HG]["partialT"].copy()
        for g in range(1, HG):
            acc += res.results[b * HG + g]["partialT"]
        out[b] = acc.T + final_bias
    return out


def _install_profile_shim():
    """Provide antenv.axon_hooks so trace=True works under axon."""
    import sys
    import types

    import antenv

    if "antenv.axon_hooks" in sys.modules:
        return
    mod = types.ModuleType("antenv.axon_hooks")
    mod._hook = None
    mod.set_axon_ntff_profile_hook = lambda h: setattr(mod, "_hook", h)
    mod.get_axon_ntff_profile_hook = lambda: mod._hook
    sys.modules["antenv.axon_hooks"] = mod
    antenv.axon_hooks = mod
    try:
        from trn_agent_boot.trn_boot import _ntff_profile_via_ctypes
        mod.set_axon_ntff_profile_hook(
            _ntff_profile_via_ctypes("/opt/axon/libaxon_pjrt.so"))
    except Exception:
        pass



# revision 7
# speedup vs baseline: 1.1602x; 1.1602x over previous
"""MultiHeadAttention Trainium2 kernel (8 NeuronCores, Bass/Tile).

Problem: B=2, S=2048, D=1024, H=16, DK=64 fp32 MHA (torch-Linear style
projections, softmax attention, output projection).

Sharding: core c = (batch b = c//4, head-group g = c%4); each core handles
4 heads of one batch in a transposed layout (features on partitions,
sequence on the free axis).

Schedule: the Scalar engine's EXP stream (128 tiles x ~1.0us) is the
critical resource; everything else is arranged to keep it saturated:
  prefix : DMA w+xk; kh projection runs kt-outer with 8 open PSUM banks so
           matmuls ride along chunk arrivals; ACT drains kh (bias+cast).
  rounds : 8 rounds (hp-major), one per (qt, hp). Round r emits, per kt:
           scores matmul pair (PE-tile-packed K=64 halves run concurrently),
           ACT exp, plus interleaved background PE work: the PV chains of
           round r-1, v-projection (rounds 0-1), lazy qh projection
           (feeds scores 1-2 rounds ahead), and oproj of finished qt.
  tail   : PV of the last round + norm + oproj(3).
Softmax denominators come free via a ones column in the PV stationary
(so Wo@bv folds into a host-side constant). No collectives; host sums 4
fp16 partials per batch.
"""

import numpy as np

B, S, D, H = 2, 2048, 1024, 16
DK = D // H          # 64
N_CORES = 8
HG = H // 4          # 4 head-groups
HL = 4               # heads per core
FEAT = HL * DK       # 256 per-core features
NQT = S // 512       # 4 query tiles
NKT = S // 128       # 16 key tiles
NDT = D // 128       # 8 contraction tiles (d-model)

_cache = {}


def _build():
    import concourse.mybir as mybir
    import concourse.tile as tile
    from concourse import bacc

    fp32 = mybir.dt.float32
    fp16 = mybir.dt.float16
    bf16 = mybir.dt.bfloat16

    nc = bacc.Bacc("TRN2", target_bir_lowering=False, debug=False,
                   num_devices=N_CORES)

    # DRAM inputs, host-prearranged so every DMA row is >=2KB contiguous
    xk_d = nc.dram_tensor("xk_d", [128, NDT, S], fp16, kind="ExternalInput").ap()
    xq_d = nc.dram_tensor("xq_d", [128, NQT, NDT, 512], fp16,
                          kind="ExternalInput").ap()
    xv_d = nc.dram_tensor("xv_d", [128, NKT, NDT, 128], fp16,
                          kind="ExternalInput").ap()
    wq_d = nc.dram_tensor("wq_d", [128, NDT, FEAT], fp16, kind="ExternalInput").ap()
    wk_d = nc.dram_tensor("wk_d", [128, NDT, FEAT], fp16, kind="ExternalInput").ap()
    wv_d = nc.dram_tensor("wv_d", [128, NDT, FEAT], fp16, kind="ExternalInput").ap()
    wo_d = nc.dram_tensor("wo_d", [128, 2, D], fp16, kind="ExternalInput").ap()
    bq_d = nc.dram_tensor("bq_d", [128, 2, 1], fp32, kind="ExternalInput").ap()
    bk_d = nc.dram_tensor("bk_d", [128, 2, 1], fp32, kind="ExternalInput").ap()
    out_d = nc.dram_tensor("partialT", [D, S], fp16, kind="ExternalOutput").ap()

    with tile.TileContext(nc) as tc:
        with (
            tc.tile_pool(name="xin", bufs=1) as xin,
            tc.tile_pool(name="win", bufs=1) as win,
            tc.tile_pool(name="proj", bufs=1) as proj,
        ):
            # ---- DMA emission order is the prefetch schedule ----
            wk3 = win.tile([128, NDT, FEAT], fp16, tag="wk")
            wq3 = win.tile([128, NDT, FEAT], fp16, tag="wq")
            wv3 = win.tile([128, NDT, FEAT], fp16, tag="wv")
            wo3 = win.tile([128, 2, D], fp16, tag="wo")
            bq3 = win.tile([128, 2, 1], fp32, tag="bq")
            bk3 = win.tile([128, 2, 1], fp32, tag="bk")
            xk3 = xin.tile([128, NDT, S], fp16, tag="xk")
            xq3 = xin.tile([128, NQT, NDT, 512], fp16, tag="xq")
            xv3 = xin.tile([128, NKT, NDT, 128], fp16, tag="xv")

            nc.sync.dma_start(wk3[:], wk_d)
            nc.sync.dma_start(wq3[:], wq_d)
            nc.sync.dma_start(wv3[:], wv_d)
            nc.sync.dma_start(bk3[:], bk_d)
            nc.sync.dma_start(bq3[:], bq_d)
            for t in range(NDT):
                nc.sync.dma_start(xk3[:, t, :], xk_d[:, t, :])
            nc.sync.dma_start(xq3[:, 0], xq_d[:, 0])
            for st in range(NKT):
                nc.sync.dma_start(xv3[:, st], xv_d[:, st])
            nc.sync.dma_start(wo3[:], wo_d)
            for n in range(1, NQT):
                nc.sync.dma_start(xq3[:, n], xq_d[:, n])

            # ---- persistent intermediates ----
            qh3 = proj.tile([128, 2, S], fp16, tag="qh")   # pair-packed
            kh3 = proj.tile([128, 2, S], fp16, tag="kh")
            vha = proj.tile([128, NKT, HL, DK + 1], bf16, tag="vha")
            ot3 = proj.tile([128, 2, S], fp16, tag="outT")

            # ---- prefix: kh projection, kt-outer over 8 PSUM banks ----
            with tc.tile_pool(name="pskh", bufs=1, space="PSUM") as pskh:
                kh8 = pskh.tile([128, 8, 512], fp32, tag="khacc")
                for kt in range(NDT):
                    for m in range(2):
                        for n in range(NQT):
                            nc.tensor.matmul(
                                kh8[:, m * 4 + n, :],
                                wk3[:, kt, m * 128:(m + 1) * 128],
                                xk3[:, kt, n * 512:(n + 1) * 512],
                                start=(kt == 0), stop=(kt == NDT - 1))
                # bias-add + fp16 cast on ACT (idle during prefix)
                for m in range(2):
                    for n in range(NQT):
                        nc.scalar.activation(
                            kh3[:, m, n * 512:(n + 1) * 512],
                            kh8[:, m * 4 + n, :],
                            mybir.ActivationFunctionType.Identity,
                            bias=bk3[:, m, :])

            with (
                tc.tile_pool(name="pexp", bufs=20) as pexp,
                tc.tile_pool(name="pout", bufs=4) as pout,
                tc.tile_pool(name="pnrm", bufs=2) as pnrm,
                tc.tile_pool(name="pp", bufs=2, space="PSUM") as pp,
                tc.tile_pool(name="ps2", bufs=2, space="PSUM") as ps2,
                tc.tile_pool(name="pspv", bufs=2, space="PSUM") as pspv,
            ):
                nc.gpsimd.memset(vha[:, :, :, DK], 1.0)  # ones column

                # ---- background work units (each ~0.4-0.9us of PE) ----
                # NOTE: units allocate their PSUM tile lazily (inside the
                # first closure) so pool slot rotation follows emission
                # order; a unit's halves are adjacent in the BG list so at
                # most 2 accumulation chains are open per pp slot pair.
                def qh_proj_half(m, n, half):
                    if half == 0:
                        _qh_ps[(m, n)] = pp.tile(
                            [128, 512], fp32, tag="acc", name=f"qacc{m}{n}")
                    ps = _qh_ps[(m, n)]
                    for kt in range(half * 4, half * 4 + 4):
                        nc.tensor.matmul(
                            ps[:], wq3[:, kt, m * 128:(m + 1) * 128],
                            xq3[:, n, kt, :],
                            start=(kt == 0), stop=(kt == NDT - 1))
                    if half == 1:
                        nc.vector.tensor_scalar_add(
                            qh3[:, m, n * 512:(n + 1) * 512], ps[:],
                            bq3[:, m, :])

                def qh_unit(m, n):
                    return [lambda: qh_proj_half(m, n, 0),
                            lambda: qh_proj_half(m, n, 1)]

                def v_half(st, half):
                    if half == 0:
                        _v_ps[st] = pp.tile(
                            [128, 256], fp32, tag="acc", name=f"vacc{st}")
                    ps = _v_ps[st]
                    for kt in range(half * 4, half * 4 + 4):
                        nc.tensor.matmul(
                            ps[:], xv3[:, st, kt, :], wv3[:, kt, :],
                            start=(kt == 0), stop=(kt == NDT - 1))
                    if half == 1:
                        nc.vector.tensor_copy(vha[:, st, :, 0:DK], ps[:])

                def v_unit(st):
                    return [lambda: v_half(st, 0), lambda: v_half(st, 1)]

                def oproj_unit(qt, jt):
                    ps = pp.tile([128, 512], fp32, tag="acc")
                    for m in range(2):
                        nc.tensor.matmul(
                            ps[:], wo3[:, m, jt * 128:(jt + 1) * 128],
                            ot3[:, m, qt * 512:(qt + 1) * 512],
                            start=(m == 0), stop=(m == 1))
                    po = pout.tile([128, 512], fp16, tag="po")
                    nc.vector.tensor_copy(po[:], ps[:])
                    nc.sync.dma_start(
                        out_d[jt * 128:(jt + 1) * 128,
                              qt * 512:(qt + 1) * 512], po[:])

                _qh_ps, _v_ps = {}, {}

                # ---- round machinery ----
                def scores_kt(qt, hp, kt):
                    s2 = ps2.tile([128, 1024], fp32, tag="s2")
                    nc.tensor.matmul(
                        s2[:, 0:512],
                        kh3[0:64, hp, kt * 128:(kt + 1) * 128],
                        qh3[0:64, hp, qt * 512:(qt + 1) * 512],
                        start=True, stop=True)
                    nc.tensor.matmul(
                        s2[:, 512:1024],
                        kh3[64:128, hp, kt * 128:(kt + 1) * 128],
                        qh3[64:128, hp, qt * 512:(qt + 1) * 512],
                        start=True, stop=True)
                    e2 = pexp.tile([128, 1024], bf16, tag="e2")
                    nc.scalar.activation(
                        e2[:], s2[:],
                        mybir.ActivationFunctionType.Exp, scale=0.125)
                    return e2

                def pv_kt(hp, e2s, kt, pva, pvb):
                    nc.tensor.matmul(
                        pva[:], vha[:, kt, 2 * hp, :], e2s[kt][:, 0:512],
                        start=(kt == 0), stop=(kt == NKT - 1))
                    nc.tensor.matmul(
                        pvb[:], vha[:, kt, 2 * hp + 1, :],
                        e2s[kt][:, 512:1024],
                        start=(kt == 0), stop=(kt == NKT - 1))

                def pv_norm(qt, hp, pva, pvb):
                    for pv, half in ((pva, 0), (pvb, 1)):
                        srow = pnrm.tile([1, 512], fp32, tag="srow")
                        nc.vector.tensor_copy(srow[:], pv[DK:DK + 1, :])
                        inv = pnrm.tile([1, 512], fp32, tag="inv")
                        nc.vector.reciprocal_approx_fast(inv[:], srow[:])
                        invb = pnrm.tile([64, 512], fp32, tag="invb")
                        nc.gpsimd.partition_broadcast(invb[:], inv[:])
                        nc.vector.tensor_tensor(
                            ot3[half * 64:(half + 1) * 64, hp,
                                qt * 512:(qt + 1) * 512],
                            pv[0:DK, :], invb[:], mybir.AluOpType.mult)

                def emit_round(qt, hp, prev, bg):
                    """One exp-bound round: 16x(scores+exp), PV of `prev`
                    interleaved, plus background units spread over slots."""
                    e2s = []
                    if prev is not None:
                        pqt, php, pe2s = prev
                        pva = pspv.tile([DK + 1, 512], fp32, tag="pv")
                        pvb = pspv.tile([DK + 1, 512], fp32, tag="pv")
                    for kt in range(NKT):
                        e2s.append(scores_kt(qt, hp, kt))
                        if prev is not None:
                            pv_kt(php, pe2s, kt, pva, pvb)
                        # drain background: spread evenly over 16 slots
                        want = ((kt + 1) * len(bg)) // NKT
                        while _bg_done[0] < want:
                            bg[_bg_done[0]]()
                            _bg_done[0] += 1
                    if prev is not None:
                        pv_norm(pqt, php, pva, pvb)
                    _bg_done[0] = 0
                    return e2s

                # ---- prefix tail: qh(0,0) ----
                for u in qh_unit(0, 0):
                    u()

                # ---- rounds, hp-major ----
                ROUNDS = [(0, 0), (1, 0), (2, 0), (3, 0),
                          (0, 1), (1, 1), (2, 1), (3, 1)]
                BG = {
                    0: [u for st in range(12) for u in v_unit(st)]
                       + qh_unit(0, 1),
                    1: [u for st in range(12, NKT) for u in v_unit(st)]
                       + qh_unit(0, 2),
                    2: qh_unit(0, 3) + qh_unit(1, 0),
                    3: qh_unit(1, 1) + qh_unit(1, 2),
                    4: qh_unit(1, 3),
                    5: [],
                    6: [lambda jt=jt: oproj_unit(0, jt) for jt in range(NDT)],
                    7: [lambda jt=jt: oproj_unit(1, jt) for jt in range(NDT)],
                }
                _bg_done = [0]
                prev = None
                for r, (qt, hp) in enumerate(ROUNDS):
                    e2s = emit_round(qt, hp, prev, BG[r])
                    prev = (qt, hp, e2s)

                # ---- tail: PV of last round interleaved with oproj(2),
                # then oproj(3).  oproj(2) can only start here: its ot3
                # half is written by pv_norm(2,1) at the end of round 7.
                pqt, php, pe2s = prev
                pva = pspv.tile([DK + 1, 512], fp32, tag="pv")
                pvb = pspv.tile([DK + 1, 512], fp32, tag="pv")
                for kt in range(NKT):
                    pv_kt(php, pe2s, kt, pva, pvb)
                    if kt % 2 == 1:
                        oproj_unit(2, kt // 2)
                pv_norm(pqt, php, pva, pvb)
                for jt in range(NDT):
                    oproj_unit(3, jt)

    nc.compile()
    return nc


def kernel(q, k, v, Wq, bq, Wk, bk, Wv, bv, Wo, bo, _trace=False):
    from concourse import bass_utils

    if "nc" not in _cache:
        _cache["nc"] = _build()
    nc = _cache["nc"]

    q = np.asarray(q, np.float32)
    k = np.asarray(k, np.float32)
    v = np.asarray(v, np.float32)
    Wq = np.asarray(Wq, np.float32)
    Wk = np.asarray(Wk, np.float32)
    Wv = np.asarray(Wv, np.float32)
    Wo = np.asarray(Wo, np.float32)
    bq = np.asarray(bq, np.float32)
    bk = np.asarray(bk, np.float32)
    bv = np.asarray(bv, np.float32)
    bo = np.asarray(bo, np.float32)

    f16 = np.float16

    # host-side pre-arrangement: all DMA rows contiguous per partition
    def arr_x_k(xT):    # [D,S] -> [128, NDT, S]
        return np.ascontiguousarray(
            xT.reshape(NDT, 128, S).transpose(1, 0, 2)).astype(f16)

    def arr_x_q(xT):    # [D,S] -> [128, NQT, NDT, 512]
        return np.ascontiguousarray(
            xT.reshape(NDT, 128, NQT, 512).transpose(1, 2, 0, 3)).astype(f16)

    def arr_x_v(xT):    # [D,S] -> [128, NKT, NDT, 128]
        return np.ascontiguousarray(
            xT.reshape(NDT, 128, NKT, 128).transpose(1, 2, 0, 3)).astype(f16)

    def arr_w(WslT):    # [D, FEAT] -> [128, NDT, FEAT]
        return np.ascontiguousarray(
            WslT.reshape(NDT, 128, FEAT).transpose(1, 0, 2)).astype(f16)

    xT = {}
    for b in range(B):
        xT[("q", b)] = arr_x_q(q[b].T)
        xT[("k", b)] = arr_x_k(k[b].T)
        xT[("v", b)] = arr_x_v(v[b].T)
    wT = {}
    for g in range(HG):
        sl = slice(g * FEAT, (g + 1) * FEAT)
        wT[("q", g)] = arr_w(Wq[sl, :].T)
        wT[("k", g)] = arr_w(Wk[sl, :].T)
        wT[("v", g)] = arr_w(Wv[sl, :].T)
        wT[("o", g)] = np.ascontiguousarray(
            Wo[:, sl].T.reshape(2, 128, D).transpose(1, 0, 2)).astype(f16)

    in_maps = []
    for c in range(N_CORES):
        b, g = divmod(c, HG)
        sl = slice(g * FEAT, (g + 1) * FEAT)
        in_maps.append({
            "xq_d": xT[("q", b)], "xk_d": xT[("k", b)], "xv_d": xT[("v", b)],
            "wq_d": wT[("q", g)], "wk_d": wT[("k", g)], "wv_d": wT[("v", g)],
            "wo_d": wT[("o", g)],
            "bq_d": np.ascontiguousarray(
                bq[sl].reshape(2, 128).T.reshape(128, 2, 1)),
            "bk_d": np.ascontiguousarray(
                bk[sl].reshape(2, 128).T.reshape(128, 2, 1)),
        })

    kwargs = {}
    if _trace:
        _install_profile_shim()
        kwargs = dict(trace=True, trace_cores=list(range(N_CORES)))
    res = bass_utils.run_bass_kernel_spmd(
        nc, in_maps, core_ids=list(range(N_CORES)), **kwargs)
    _cache["last_results"] = res

    final_bias = (Wo @ bv + bo).astype(np.float32)  # attn rows sum to 1
    out = np.empty((B, S, D), np.float32)
    for b in range(B):
        acc = res.results[b * HG]["partialT"].astype(np.float32)
        for g in range(1, HG):
            acc += res.results[b * HG + g]["partialT"].astype(np.float32)
        out[b] = acc.T + final_bias
    return out


def _install_profile_shim():
    """Provide antenv.axon_hooks so trace=True works under axon."""
    import sys
    import types

    import antenv

    if "antenv.axon_hooks" in sys.modules:
        return
    mod = types.ModuleType("antenv.axon_hooks")
    mod._hook = None
    mod.set_axon_ntff_profile_hook = lambda h: setattr(mod, "_hook", h)
    mod.get_axon_ntff_profile_hook = lambda: mod._hook
    sys.modules["antenv.axon_hooks"] = mod
    antenv.axon_hooks = mod
    try:
        from trn_agent_boot.trn_boot import _ntff_profile_via_ctypes
        mod.set_axon_ntff_profile_hook(
            _ntff_profile_via_ctypes("/opt/axon/libaxon_pjrt.so"))
    except Exception:
        pass


# revision 15
# speedup vs baseline: 1.2071x; 1.0405x over previous
"""MultiHeadAttention Trainium2 kernel (8 NeuronCores, Bass/Tile).

Problem: B=2, S=2048, D=1024, H=16, DK=64 fp32 MHA (torch-Linear style
projections, softmax attention, output projection).

Sharding: core c = (batch b = c//4, head-group g = c%4); each core handles
4 heads of one batch in a transposed layout (features on partitions,
sequence on the free axis).

Schedule: the Scalar engine's EXP stream (128 tiles x ~1.0us) is the
critical resource; everything else is arranged to keep it saturated:
  prefix : DMA w+xk; kh projection runs kt-outer with 8 open PSUM banks so
           matmuls ride along chunk arrivals; ACT drains kh (bias+cast).
  rounds : 8 rounds (hp-major), one per (qt, hp). Round r emits, per kt:
           scores matmul pair (PE-tile-packed K=64 halves run concurrently),
           ACT exp, plus interleaved background PE work: the PV chains of
           round r-1, v-projection (rounds 0-1), lazy qh projection
           (feeds scores 1-2 rounds ahead), and oproj of finished qt.
  tail   : PV of the last round + norm + oproj(3).
Softmax denominators come free via a ones column in the PV stationary
(so Wo@bv folds into a host-side constant). No collectives; host sums 4
fp16 partials per batch.
"""

import numpy as np

B, S, D, H = 2, 2048, 1024, 16
DK = D // H          # 64
N_CORES = 8
HG = H // 4          # 4 head-groups
HL = 4               # heads per core
FEAT = HL * DK       # 256 per-core features
NQT = S // 512       # 4 query tiles
NKT = S // 128       # 16 key tiles
NDT = D // 128       # 8 contraction tiles (d-model)

_cache = {}


def _build():
    import concourse.mybir as mybir
    import concourse.tile as tile
    from concourse import bacc

    fp32 = mybir.dt.float32
    fp16 = mybir.dt.float16
    bf16 = mybir.dt.bfloat16

    nc = bacc.Bacc("TRN2", target_bir_lowering=False, debug=False,
                   num_devices=N_CORES)

    # DRAM inputs, host-prearranged so every DMA row is >=2KB contiguous
    xk_d = nc.dram_tensor("xk_d", [128, NDT, S], fp16, kind="ExternalInput").ap()
    xq_d = nc.dram_tensor("xq_d", [128, NQT, NDT, 512], fp16,
                          kind="ExternalInput").ap()
    xv_d = nc.dram_tensor("xv_d", [128, NKT, NDT, 128], fp16,
                          kind="ExternalInput").ap()
    wq_d = nc.dram_tensor("wq_d", [128, NDT, FEAT], fp16, kind="ExternalInput").ap()
    wk_d = nc.dram_tensor("wk_d", [128, NDT, FEAT], fp16, kind="ExternalInput").ap()
    wv_d = nc.dram_tensor("wv_d", [128, NDT, FEAT], fp16, kind="ExternalInput").ap()
    wo_d = nc.dram_tensor("wo_d", [128, 2, D], fp16, kind="ExternalInput").ap()
    bq_d = nc.dram_tensor("bq_d", [128, 2, 1], fp32, kind="ExternalInput").ap()
    bk_d = nc.dram_tensor("bk_d", [128, 2, 1], fp32, kind="ExternalInput").ap()
    out_d = nc.dram_tensor("partialT", [D, S], fp16, kind="ExternalOutput").ap()

    with tile.TileContext(nc) as tc:
        with (
            tc.tile_pool(name="xin", bufs=1) as xin,
            tc.tile_pool(name="win", bufs=1) as win,
            tc.tile_pool(name="proj", bufs=1) as proj,
        ):
            # ---- DMA emission order is the prefetch schedule ----
            wk3 = win.tile([128, NDT, FEAT], fp16, tag="wk")
            wq3 = win.tile([128, NDT, FEAT], fp16, tag="wq")
            wv3 = win.tile([128, NDT, FEAT], fp16, tag="wv")
            wo3 = win.tile([128, 2, D], fp16, tag="wo")
            bq3 = win.tile([128, 2, 1], fp32, tag="bq")
            bk3 = win.tile([128, 2, 1], fp32, tag="bk")
            xk3 = xin.tile([128, NDT, S], fp16, tag="xk")
            xq3 = xin.tile([128, NQT, NDT, 512], fp16, tag="xq")
            xv3 = xin.tile([128, NKT, NDT, 128], fp16, tag="xv")

            nc.sync.dma_start(wk3[:], wk_d)
            nc.sync.dma_start(wq3[:], wq_d)
            nc.sync.dma_start(wv3[:], wv_d)
            nc.sync.dma_start(bk3[:], bk_d)
            nc.sync.dma_start(bq3[:], bq_d)
            for t in range(NDT):
                nc.sync.dma_start(xk3[:, t, :], xk_d[:, t, :])
            nc.sync.dma_start(xq3[:, 0], xq_d[:, 0])
            for st in range(8):
                nc.sync.dma_start(xv3[:, st], xv_d[:, st])
            nc.sync.dma_start(xq3[:, 1], xq_d[:, 1])
            for st in range(8, NKT):
                nc.sync.dma_start(xv3[:, st], xv_d[:, st])
            nc.sync.dma_start(wo3[:], wo_d)
            nc.sync.dma_start(xq3[:, 2], xq_d[:, 2])
            nc.sync.dma_start(xq3[:, 3], xq_d[:, 3])

            # ---- persistent intermediates ----
            qh3 = proj.tile([128, 2, S], fp16, tag="qh")   # pair-packed
            kh3 = proj.tile([128, 2, S], fp16, tag="kh")
            vha = proj.tile([128, NKT, HL, DK + 1], bf16, tag="vha")
            ot3 = proj.tile([128, 2, S], fp16, tag="outT")

            # ---- prefix: kh projection, kt-outer over 8 PSUM banks ----
            with tc.tile_pool(name="pskh", bufs=1, space="PSUM") as pskh:
                kh8 = pskh.tile([128, 8, 512], fp32, tag="khacc")
                for kt in range(NDT):
                    for m in range(2):
                        for n in range(NQT):
                            nc.tensor.matmul(
                                kh8[:, m * 4 + n, :],
                                wk3[:, kt, m * 128:(m + 1) * 128],
                                xk3[:, kt, n * 512:(n + 1) * 512],
                                start=(kt == 0), stop=(kt == NDT - 1))
                # bias-add + fp16 cast on ACT (idle during prefix)
                for m in range(2):
                    for n in range(NQT):
                        nc.scalar.activation(
                            kh3[:, m, n * 512:(n + 1) * 512],
                            kh8[:, m * 4 + n, :],
                            mybir.ActivationFunctionType.Identity,
                            bias=bk3[:, m, :])

            with (
                tc.tile_pool(name="pexp", bufs=20) as pexp,
                tc.tile_pool(name="pout", bufs=4) as pout,
                tc.tile_pool(name="pnrm", bufs=2) as pnrm,
                tc.tile_pool(name="pp", bufs=2, space="PSUM") as pp,
                tc.tile_pool(name="ps2", bufs=2, space="PSUM") as ps2,
                tc.tile_pool(name="pspv", bufs=2, space="PSUM") as pspv,
            ):
                nc.gpsimd.memset(vha[:, :, :, DK], 1.0)  # ones column

                # ---- background work units (each ~0.4-0.9us of PE) ----
                # NOTE: units allocate their PSUM tile lazily (inside the
                # first closure) so pool slot rotation follows emission
                # order; a unit's halves are adjacent in the BG list so at
                # most 2 accumulation chains are open per pp slot pair.
                def qh_proj_half(m, n, half):
                    if half == 0:
                        _qh_ps[(m, n)] = pp.tile(
                            [128, 512], fp32, tag="acc", name=f"qacc{m}{n}")
                    ps = _qh_ps[(m, n)]
                    for kt in range(half * 4, half * 4 + 4):
                        nc.tensor.matmul(
                            ps[:], wq3[:, kt, m * 128:(m + 1) * 128],
                            xq3[:, n, kt, :],
                            start=(kt == 0), stop=(kt == NDT - 1))
                    if half == 1:
                        nc.vector.tensor_scalar_add(
                            qh3[:, m, n * 512:(n + 1) * 512], ps[:],
                            bq3[:, m, :])

                def qh_unit(m, n):
                    return [lambda: qh_proj_half(m, n, 0),
                            lambda: qh_proj_half(m, n, 1)]

                def v_half(st, half):
                    if half == 0:
                        _v_ps[st] = pp.tile(
                            [128, 256], fp32, tag="acc", name=f"vacc{st}")
                    ps = _v_ps[st]
                    for kt in range(half * 4, half * 4 + 4):
                        nc.tensor.matmul(
                            ps[:], xv3[:, st, kt, :], wv3[:, kt, :],
                            start=(kt == 0), stop=(kt == NDT - 1))
                    if half == 1:
                        nc.vector.tensor_copy(vha[:, st, :, 0:DK], ps[:])

                def v_unit(st):
                    return [lambda: v_half(st, 0), lambda: v_half(st, 1)]

                def oproj_unit(qt, jt, on_act=False):
                    ps = pp.tile([128, 512], fp32, tag="acc")
                    for m in range(2):
                        nc.tensor.matmul(
                            ps[:], wo3[:, m, jt * 128:(jt + 1) * 128],
                            ot3[:, m, qt * 512:(qt + 1) * 512],
                            start=(m == 0), stop=(m == 1))
                    po = pout.tile([128, 512], fp16, tag="po")
                    if on_act:   # tail: ACT is idle, DVE is the bottleneck
                        nc.scalar.copy(po[:], ps[:])
                    else:
                        nc.vector.tensor_copy(po[:], ps[:])
                    nc.sync.dma_start(
                        out_d[jt * 128:(jt + 1) * 128,
                              qt * 512:(qt + 1) * 512], po[:])

                _qh_ps, _v_ps = {}, {}

                # ---- round machinery ----
                def scores_kt(qt, hp, kt):
                    s2 = ps2.tile([128, 1024], fp32, tag="s2")
                    nc.tensor.matmul(
                        s2[:, 0:512],
                        kh3[0:64, hp, kt * 128:(kt + 1) * 128],
                        qh3[0:64, hp, qt * 512:(qt + 1) * 512],
                        start=True, stop=True)
                    nc.tensor.matmul(
                        s2[:, 512:1024],
                        kh3[64:128, hp, kt * 128:(kt + 1) * 128],
                        qh3[64:128, hp, qt * 512:(qt + 1) * 512],
                        start=True, stop=True)
                    e2 = pexp.tile([128, 1024], bf16, tag="e2")
                    nc.scalar.activation(
                        e2[:], s2[:],
                        mybir.ActivationFunctionType.Exp, scale=0.125)
                    return e2

                def pv_kt(hp, e2s, kt, pva, pvb):
                    nc.tensor.matmul(
                        pva[:], vha[:, kt, 2 * hp, :], e2s[kt][:, 0:512],
                        start=(kt == 0), stop=(kt == NKT - 1))
                    nc.tensor.matmul(
                        pvb[:], vha[:, kt, 2 * hp + 1, :],
                        e2s[kt][:, 512:1024],
                        start=(kt == 0), stop=(kt == NKT - 1))

                def pv_norm(qt, hp, pva, pvb):
                    for pv, half in ((pva, 0), (pvb, 1)):
                        srow = pnrm.tile([1, 512], fp32, tag="srow")
                        nc.vector.tensor_copy(srow[:], pv[DK:DK + 1, :])
                        inv = pnrm.tile([1, 512], fp32, tag="inv")
                        nc.vector.reciprocal_approx_fast(inv[:], srow[:])
                        invb = pnrm.tile([64, 512], fp32, tag="invb")
                        nc.gpsimd.partition_broadcast(invb[:], inv[:])
                        nc.vector.tensor_tensor(
                            ot3[half * 64:(half + 1) * 64, hp,
                                qt * 512:(qt + 1) * 512],
                            pv[0:DK, :], invb[:], mybir.AluOpType.mult)

                def emit_round(qt, hp, prev, bg, bg_front=True):
                    """One exp-bound round: 16x(scores+exp), PV of `prev`
                    interleaved at 1.5 kt/slot (finishes ~slot 11 so its
                    norm frees the pspv slots before the NEXT round's PV
                    starts), plus background units spread over slots.

                    Background drains BEFORE the PV chain in each slot and
                    (with bg_front) at >=1 unit/slot, because emission
                    order IS dataflow: a unit whose output a later PV
                    matmul reads (v-proj feeding vha) must be emitted
                    before it."""
                    e2s = []
                    pv_done = 0
                    if prev is not None:
                        pqt, php, pe2s = prev
                        pva = pspv.tile([DK + 1, 512], fp32, tag="pv")
                        pvb = pspv.tile([DK + 1, 512], fp32, tag="pv")
                    for kt in range(NKT):
                        e2s.append(scores_kt(qt, hp, kt))
                        want = ((kt + 1) * len(bg)) // NKT
                        if bg_front:
                            want = max(want, min(kt + 1, len(bg)))
                        while _bg_done[0] < want:
                            bg[_bg_done[0]]()
                            _bg_done[0] += 1
                        if prev is not None:
                            pv_want = min(NKT, ((kt + 1) * 3 + 1) // 2)
                            while pv_done < pv_want:
                                pv_kt(php, pe2s, pv_done, pva, pvb)
                                pv_done += 1
                            if pv_done == NKT:
                                pv_norm(pqt, php, pva, pvb)
                                pv_done += 1  # emit norm once
                    _bg_done[0] = 0
                    return e2s

                # ---- prefix tail: qh(0,0) ----
                for u in qh_unit(0, 0):
                    u()

                # ---- rounds, hp-major ----
                ROUNDS = [(0, 0), (1, 0), (2, 0), (3, 0),
                          (0, 1), (1, 1), (2, 1), (3, 1)]
                # qh units sit early/mid round so their bias-add lands well
                # before the next round's first scores matmul needs them.
                BG = {
                    0: [u for st in range(8) for u in v_unit(st)]
                       + qh_unit(0, 1)
                       + [u for st in range(8, 12) for u in v_unit(st)],
                    1: [u for st in range(12, NKT) for u in v_unit(st)]
                       + qh_unit(0, 2),
                    2: qh_unit(0, 3) + qh_unit(1, 0),
                    3: qh_unit(1, 1) + qh_unit(1, 2),
                    4: qh_unit(1, 3),
                    5: [],
                    6: [lambda jt=jt: oproj_unit(0, jt) for jt in range(NDT)],
                    7: [lambda jt=jt: oproj_unit(1, jt) for jt in range(NDT)],
                }
                _bg_done = [0]
                prev = None
                for r, (qt, hp) in enumerate(ROUNDS):
                    e2s = emit_round(qt, hp, prev, BG[r], bg_front=(r < 6))
                    prev = (qt, hp, e2s)

                # ---- tail: PV of last round interleaved with oproj(2),
                # then oproj(3).  oproj(2) can only start here: its ot3
                # half is written by pv_norm(2,1) at the end of round 7.
                pqt, php, pe2s = prev
                pva = pspv.tile([DK + 1, 512], fp32, tag="pv")
                pvb = pspv.tile([DK + 1, 512], fp32, tag="pv")
                for kt in range(NKT):
                    pv_kt(php, pe2s, kt, pva, pvb)
                    if kt % 2 == 1:
                        oproj_unit(2, kt // 2, on_act=True)
                pv_norm(pqt, php, pva, pvb)
                for jt in range(NDT):
                    oproj_unit(3, jt, on_act=True)

    nc.compile()
    return nc


def kernel(q, k, v, Wq, bq, Wk, bk, Wv, bv, Wo, bo, _trace=False):
    from concourse import bass_utils

    if "nc" not in _cache:
        _cache["nc"] = _build()
    nc = _cache["nc"]

    q = np.asarray(q, np.float32)
    k = np.asarray(k, np.float32)
    v = np.asarray(v, np.float32)
    Wq = np.asarray(Wq, np.float32)
    Wk = np.asarray(Wk, np.float32)
    Wv = np.asarray(Wv, np.float32)
    Wo = np.asarray(Wo, np.float32)
    bq = np.asarray(bq, np.float32)
    bk = np.asarray(bk, np.float32)
    bv = np.asarray(bv, np.float32)
    bo = np.asarray(bo, np.float32)

    f16 = np.float16

    # host-side pre-arrangement: all DMA rows contiguous per partition
    def arr_x_k(xT):    # [D,S] -> [128, NDT, S]
        return np.ascontiguousarray(
            xT.reshape(NDT, 128, S).transpose(1, 0, 2)).astype(f16)

    def arr_x_q(xT):    # [D,S] -> [128, NQT, NDT, 512]
        return np.ascontiguousarray(
            xT.reshape(NDT, 128, NQT, 512).transpose(1, 2, 0, 3)).astype(f16)

    def arr_x_v(xT):    # [D,S] -> [128, NKT, NDT, 128]
        return np.ascontiguousarray(
            xT.reshape(NDT, 128, NKT, 128).transpose(1, 2, 0, 3)).astype(f16)

    def arr_w(WslT):    # [D, FEAT] -> [128, NDT, FEAT]
        return np.ascontiguousarray(
            WslT.reshape(NDT, 128, FEAT).transpose(1, 0, 2)).astype(f16)

    xT = {}
    for b in range(B):
        xT[("q", b)] = arr_x_q(q[b].T)
        xT[("k", b)] = arr_x_k(k[b].T)
        xT[("v", b)] = arr_x_v(v[b].T)
    wT = {}
    for g in range(HG):
        sl = slice(g * FEAT, (g + 1) * FEAT)
        wT[("q", g)] = arr_w(Wq[sl, :].T)
        wT[("k", g)] = arr_w(Wk[sl, :].T)
        wT[("v", g)] = arr_w(Wv[sl, :].T)
        wT[("o", g)] = np.ascontiguousarray(
            Wo[:, sl].T.reshape(2, 128, D).transpose(1, 0, 2)).astype(f16)

    in_maps = []
    for c in range(N_CORES):
        b, g = divmod(c, HG)
        sl = slice(g * FEAT, (g + 1) * FEAT)
        in_maps.append({
            "xq_d": xT[("q", b)], "xk_d": xT[("k", b)], "xv_d": xT[("v", b)],
            "wq_d": wT[("q", g)], "wk_d": wT[("k", g)], "wv_d": wT[("v", g)],
            "wo_d": wT[("o", g)],
            "bq_d": np.ascontiguousarray(
                bq[sl].reshape(2, 128).T.reshape(128, 2, 1)),
            "bk_d": np.ascontiguousarray(
                bk[sl].reshape(2, 128).T.reshape(128, 2, 1)),
        })

    kwargs = {}
    if _trace:
        _install_profile_shim()
        kwargs = dict(trace=True, trace_cores=list(range(N_CORES)))
    res = bass_utils.run_bass_kernel_spmd(
        nc, in_maps, core_ids=list(range(N_CORES)), **kwargs)
    _cache["last_results"] = res

    final_bias = (Wo @ bv + bo).astype(np.float32)  # attn rows sum to 1
    out = np.empty((B, S, D), np.float32)
    for b in range(B):
        acc = res.results[b * HG]["partialT"].astype(np.float32)
        for g in range(1, HG):
            acc += res.results[b * HG + g]["partialT"].astype(np.float32)
        out[b] = acc.T + final_bias
    return out


def _install_profile_shim():
    """Provide antenv.axon_hooks so trace=True works under axon."""
    import sys
    import types

    import antenv

    if "antenv.axon_hooks" in sys.modules:
        return
    mod = types.ModuleType("antenv.axon_hooks")
    mod._hook = None
    mod.set_axon_ntff_profile_hook = lambda h: setattr(mod, "_hook", h)
    mod.get_axon_ntff_profile_hook = lambda: mod._hook
    sys.modules["antenv.axon_hooks"] = mod
    antenv.axon_hooks = mod
    try:
        from trn_agent_boot.trn_boot import _ntff_profile_via_ctypes
        mod.set_axon_ntff_profile_hook(
            _ntff_profile_via_ctypes("/opt/axon/libaxon_pjrt.so"))
    except Exception:
        pass


# revision 21
# speedup vs baseline: 1.2447x; 1.0312x over previous
"""MultiHeadAttention Trainium2 kernel (8 NeuronCores, Bass/Tile).

Problem: B=2, S=2048, D=1024, H=16, DK=64 fp32 MHA (torch-Linear style
projections, softmax attention, output projection).

Sharding: core c = (batch b = c//4, head-group g = c%4); each core handles
4 heads of one batch in a transposed layout (features on partitions,
sequence on the free axis).

Schedule: the Scalar engine's EXP stream (128 tiles x ~1.0us) is the
critical resource; everything else is arranged to keep it saturated:
  prefix : DMA w + xk/xq first column-block; project kh(m0,n0)+qh(m0,n0)
           only, so the first scores matmul fires ~3MB into the input
           stream instead of after all of xk.
  rounds : 8 rounds (hp-major), one per (qt, hp). Round r emits, per kt
           slot: scores matmul pair (PE-tile-packed K=64 halves run
           concurrently), ACT exp, the PV chain of round r-1 paced evenly,
           and background PE units on an explicit per-slot schedule:
           remaining kh/qh projection column-blocks (each feeding scores
           1-4 rounds ahead), v-projection (feeding round-1 PV), and
           oproj of finished qt.  Emission order IS dataflow, so every
           unit is placed before its first consumer.
  tail   : PV of the last round interleaved with oproj(2), then oproj(3)
           with PSUM->SBUF copies alternating ACT/DVE.
Softmax denominators come free via a ones column in the PV stationary
(so Wo@bv folds into a host-side constant). No collectives; host sums 4
fp16 partials per batch.
"""

import numpy as np

B, S, D, H = 2, 2048, 1024, 16
DK = D // H          # 64
N_CORES = 8
HG = H // 4          # 4 head-groups
HL = 4               # heads per core
FEAT = HL * DK       # 256 per-core features
NQT = S // 512       # 4 query tiles
NKT = S // 128       # 16 key tiles
NDT = D // 128       # 8 contraction tiles (d-model)

_cache = {}


def _build():
    import concourse.mybir as mybir
    import concourse.tile as tile
    from concourse import bacc

    fp32 = mybir.dt.float32
    fp16 = mybir.dt.float16
    bf16 = mybir.dt.bfloat16

    nc = bacc.Bacc("TRN2", target_bir_lowering=False, debug=False,
                   num_devices=N_CORES)

    # DRAM inputs, host-prearranged so every DMA row is >=2KB contiguous
    xk_d = nc.dram_tensor("xk_d", [128, NQT, NDT, 512], fp16,
                          kind="ExternalInput").ap()
    xq_d = nc.dram_tensor("xq_d", [128, NQT, NDT, 512], fp16,
                          kind="ExternalInput").ap()
    xv_d = nc.dram_tensor("xv_d", [128, NKT, NDT, 128], fp16,
                          kind="ExternalInput").ap()
    wq_d = nc.dram_tensor("wq_d", [128, NDT, FEAT], fp16, kind="ExternalInput").ap()
    wk_d = nc.dram_tensor("wk_d", [128, NDT, FEAT], fp16, kind="ExternalInput").ap()
    wv_d = nc.dram_tensor("wv_d", [128, NDT, FEAT], fp16, kind="ExternalInput").ap()
    wo_d = nc.dram_tensor("wo_d", [128, 2, D], fp16, kind="ExternalInput").ap()
    bq_d = nc.dram_tensor("bq_d", [128, 2, 1], fp32, kind="ExternalInput").ap()
    bk_d = nc.dram_tensor("bk_d", [128, 2, 1], fp32, kind="ExternalInput").ap()
    out_d = nc.dram_tensor("partialT", [D, S], fp16, kind="ExternalOutput").ap()

    with tile.TileContext(nc) as tc:
        with (
            tc.tile_pool(name="xin", bufs=1) as xin,
            tc.tile_pool(name="win", bufs=1) as win,
            tc.tile_pool(name="proj", bufs=1) as proj,
        ):
            # ---- DMA emission order is the prefetch schedule ----
            wk3 = win.tile([128, NDT, FEAT], fp16, tag="wk")
            wq3 = win.tile([128, NDT, FEAT], fp16, tag="wq")
            wv3 = win.tile([128, NDT, FEAT], fp16, tag="wv")
            wo3 = win.tile([128, 2, D], fp16, tag="wo")
            bq3 = win.tile([128, 2, 1], fp32, tag="bq")
            bk3 = win.tile([128, 2, 1], fp32, tag="bk")
            xk3 = xin.tile([128, NQT, NDT, 512], fp16, tag="xk")
            xq3 = xin.tile([128, NQT, NDT, 512], fp16, tag="xq")
            xv3 = xin.tile([128, NKT, NDT, 128], fp16, tag="xv")

            nc.sync.dma_start(wk3[:], wk_d)
            nc.sync.dma_start(wq3[:], wq_d)
            nc.sync.dma_start(bk3[:], bk_d)
            nc.sync.dma_start(bq3[:], bq_d)
            nc.sync.dma_start(xk3[:, 0], xk_d[:, 0])
            nc.sync.dma_start(xq3[:, 0], xq_d[:, 0])
            nc.sync.dma_start(xk3[:, 1], xk_d[:, 1])
            nc.sync.dma_start(xq3[:, 1], xq_d[:, 1])
            nc.sync.dma_start(xk3[:, 2], xk_d[:, 2])
            nc.sync.dma_start(xk3[:, 3], xk_d[:, 3])
            nc.sync.dma_start(wv3[:], wv_d)
            for st in range(NKT):
                nc.sync.dma_start(xv3[:, st], xv_d[:, st])
            nc.sync.dma_start(wo3[:], wo_d)
            nc.sync.dma_start(xq3[:, 2], xq_d[:, 2])
            nc.sync.dma_start(xq3[:, 3], xq_d[:, 3])

            # ---- persistent intermediates ----
            qh3 = proj.tile([128, 2, S], fp16, tag="qh")   # pair-packed
            kh3 = proj.tile([128, 2, S], fp16, tag="kh")
            vha = proj.tile([128, NKT, HL, DK + 1], bf16, tag="vha")
            ot3 = proj.tile([128, 2, S], fp16, tag="outT")

            # ---- prefix: kh(m0,n0) + qh(m0,n0) only ----
            with tc.tile_pool(name="pskh", bufs=1, space="PSUM") as pskh:
                kacc = pskh.tile([128, 512], fp32, tag="kacc")
                for kt in range(NDT):
                    nc.tensor.matmul(
                        kacc[:], wk3[:, kt, 0:128], xk3[:, 0, kt, :],
                        start=(kt == 0), stop=(kt == NDT - 1))
                nc.vector.tensor_scalar_add(
                    kh3[:, 0, 0:512], kacc[:], bk3[:, 0, :])
                qacc = pskh.tile([128, 512], fp32, tag="qacc")
                for kt in range(NDT):
                    nc.tensor.matmul(
                        qacc[:], wq3[:, kt, 0:128], xq3[:, 0, kt, :],
                        start=(kt == 0), stop=(kt == NDT - 1))
                nc.vector.tensor_scalar_add(
                    qh3[:, 0, 0:512], qacc[:], bq3[:, 0, :])

            with (
                tc.tile_pool(name="pexp", bufs=20) as pexp,
                tc.tile_pool(name="pout", bufs=4) as pout,
                tc.tile_pool(name="pnrm", bufs=2) as pnrm,
                tc.tile_pool(name="pp", bufs=2, space="PSUM") as pp,
                tc.tile_pool(name="ps2", bufs=2, space="PSUM") as ps2,
                tc.tile_pool(name="pspv", bufs=2, space="PSUM") as pspv,
            ):
                nc.gpsimd.memset(vha[:, :, :, DK], 1.0)  # ones column

                # ---- background work units (~0.4-0.9us of PE each) ----
                # Units allocate their PSUM tile lazily (inside the first
                # closure) so pool slot rotation follows emission order; a
                # unit's halves are adjacent so at most 2 accumulation
                # chains are open per pp slot pair.
                _acc = {}

                def proj_half(key, w3, b3, x3, dst, m, n, half):
                    if half == 0:
                        _acc[(key, m, n)] = pp.tile(
                            [128, 512], fp32, tag="acc",
                            name=f"pa_{key}{m}{n}")
                    ps = _acc[(key, m, n)]
                    for kt in range(half * 4, half * 4 + 4):
                        nc.tensor.matmul(
                            ps[:], w3[:, kt, m * 128:(m + 1) * 128],
                            x3[:, n, kt, :],
                            start=(kt == 0), stop=(kt == NDT - 1))
                    if half == 1:
                        nc.vector.tensor_scalar_add(
                            dst[:, m, n * 512:(n + 1) * 512], ps[:],
                            b3[:, m, :])

                def qh_unit(m, n, slots):
                    return [(slots[0], lambda: proj_half(
                                "q", wq3, bq3, xq3, qh3, m, n, 0)),
                            (slots[1], lambda: proj_half(
                                "q", wq3, bq3, xq3, qh3, m, n, 1))]

                def kh_unit(m, n, slots):
                    return [(slots[0], lambda: proj_half(
                                "k", wk3, bk3, xk3, kh3, m, n, 0)),
                            (slots[1], lambda: proj_half(
                                "k", wk3, bk3, xk3, kh3, m, n, 1))]

                def v_half(st, half):
                    if half == 0:
                        _acc[("v", st)] = pp.tile(
                            [128, 256], fp32, tag="acc", name=f"vacc{st}")
                    ps = _acc[("v", st)]
                    for kt in range(half * 4, half * 4 + 4):
                        nc.tensor.matmul(
                            ps[:], xv3[:, st, kt, :], wv3[:, kt, :],
                            start=(kt == 0), stop=(kt == NDT - 1))
                    if half == 1:
                        nc.vector.tensor_copy(vha[:, st, :, 0:DK], ps[:])

                def v_unit(st, slots):
                    return [(slots[0], lambda: v_half(st, 0)),
                            (slots[1], lambda: v_half(st, 1))]

                def oproj_unit(qt, jt, on_act=False):
                    ps = pp.tile([128, 512], fp32, tag="acc")
                    for m in range(2):
                        nc.tensor.matmul(
                            ps[:], wo3[:, m, jt * 128:(jt + 1) * 128],
                            ot3[:, m, qt * 512:(qt + 1) * 512],
                            start=(m == 0), stop=(m == 1))
                    po = pout.tile([128, 512], fp16, tag="po")
                    if on_act:   # tail: ACT is idle, DVE is busier
                        nc.scalar.copy(po[:], ps[:])
                    else:
                        nc.vector.tensor_copy(po[:], ps[:])
                    nc.sync.dma_start(
                        out_d[jt * 128:(jt + 1) * 128,
                              qt * 512:(qt + 1) * 512], po[:])

                def oproj_units(qt, slots):
                    return [(s, lambda jt=jt: oproj_unit(qt, jt))
                            for jt, s in enumerate(slots)]

                # ---- round machinery ----
                def scores_kt(qt, hp, kt):
                    s2 = ps2.tile([128, 1024], fp32, tag="s2")
                    nc.tensor.matmul(
                        s2[:, 0:512],
                        kh3[0:64, hp, kt * 128:(kt + 1) * 128],
                        qh3[0:64, hp, qt * 512:(qt + 1) * 512],
                        start=True, stop=True)
                    nc.tensor.matmul(
                        s2[:, 512:1024],
                        kh3[64:128, hp, kt * 128:(kt + 1) * 128],
                        qh3[64:128, hp, qt * 512:(qt + 1) * 512],
                        start=True, stop=True)
                    e2 = pexp.tile([128, 1024], bf16, tag="e2")
                    nc.scalar.activation(
                        e2[:], s2[:],
                        mybir.ActivationFunctionType.Exp, scale=0.125)
                    return e2

                def pv_kt(hp, e2s, kt, pva, pvb):
                    nc.tensor.matmul(
                        pva[:], vha[:, kt, 2 * hp, :], e2s[kt][:, 0:512],
                        start=(kt == 0), stop=(kt == NKT - 1))
                    nc.tensor.matmul(
                        pvb[:], vha[:, kt, 2 * hp + 1, :],
                        e2s[kt][:, 512:1024],
                        start=(kt == 0), stop=(kt == NKT - 1))

                def pv_norm(qt, hp, pva, pvb):
                    for pv, half in ((pva, 0), (pvb, 1)):
                        srow = pnrm.tile([1, 512], fp32, tag="srow")
                        nc.vector.tensor_copy(srow[:], pv[DK:DK + 1, :])
                        inv = pnrm.tile([1, 512], fp32, tag="inv")
                        nc.vector.reciprocal_approx_fast(inv[:], srow[:])
                        invb = pnrm.tile([64, 512], fp32, tag="invb")
                        nc.gpsimd.partition_broadcast(invb[:], inv[:])
                        nc.vector.tensor_tensor(
                            ot3[half * 64:(half + 1) * 64, hp,
                                qt * 512:(qt + 1) * 512],
                            pv[0:DK, :], invb[:], mybir.AluOpType.mult)

                def emit_round(qt, hp, prev, bg):
                    """One exp-bound round.  Per kt slot: scores+exp, then
                    background units whose scheduled slot has arrived (in
                    list order — emission order IS dataflow), then the PV
                    chain of `prev` paced ~9/8 kt per slot with its norm
                    right after the last chain matmul."""
                    e2s = []
                    pv_done = 0
                    if prev is not None:
                        pqt, php, pe2s = prev
                        pva = pspv.tile([DK + 1, 512], fp32, tag="pv")
                        pvb = pspv.tile([DK + 1, 512], fp32, tag="pv")
                    for kt in range(NKT):
                        e2s.append(scores_kt(qt, hp, kt))
                        while _bg_done[0] < len(bg) and \
                                bg[_bg_done[0]][0] <= kt:
                            bg[_bg_done[0]][1]()
                            _bg_done[0] += 1
                        if prev is not None:
                            pv_want = min(NKT + 1, ((kt + 1) * 5) // 4)
                            while pv_done < pv_want:
                                if pv_done < NKT:
                                    pv_kt(php, pe2s, pv_done, pva, pvb)
                                else:
                                    pv_norm(pqt, php, pva, pvb)
                                pv_done += 1
                    while _bg_done[0] < len(bg):   # drain leftovers
                        bg[_bg_done[0]][1]()
                        _bg_done[0] += 1
                    if prev is not None and pv_done <= NKT:
                        while pv_done < NKT:
                            pv_kt(php, pe2s, pv_done, pva, pvb)
                            pv_done += 1
                        pv_norm(pqt, php, pva, pvb)
                    _bg_done[0] = 0
                    return e2s

                # ---- rounds, hp-major ----
                # Background placement is deadline-driven:
                #   kh(0,n) before scores kt=4n of the SAME round 0;
                #   qh(m,n) one round before scores(qt=n, hp=m);
                #   kh(1,n) any time before round 4;
                #   v(st) before PV(0,0) kt=st in round 1;
                #   oproj(qt) after pv_norm(qt, 1).
                ROUNDS = [(0, 0), (1, 0), (2, 0), (3, 0),
                          (0, 1), (1, 1), (2, 1), (3, 1)]
                BG = {
                    0: kh_unit(0, 1, (2, 3)) + qh_unit(0, 1, (5, 6))
                       + kh_unit(0, 2, (6, 7)) + kh_unit(0, 3, (10, 11))
                       + v_unit(0, (13, 13)) + v_unit(1, (14, 15)),
                    1: [u for j in range(2, NKT)
                        for u in v_unit(j, (max(0, j - 5), max(0, j - 4)))]
                       + qh_unit(0, 2, (11, 12)),
                    2: qh_unit(0, 3, (0, 1)) + kh_unit(1, 0, (2, 3))
                       + kh_unit(1, 1, (5, 6)),
                    3: qh_unit(1, 0, (0, 1)) + kh_unit(1, 2, (2, 3))
                       + kh_unit(1, 3, (5, 6)),
                    4: qh_unit(1, 1, (0, 1)),
                    5: qh_unit(1, 2, (0, 1)),
                    6: qh_unit(1, 3, (0, 1))
                       + oproj_units(0, (2, 4, 6, 8, 10, 12, 13, 14)),
                    7: oproj_units(1, (1, 3, 5, 7, 9, 11, 13, 14)),
                }
                _bg_done = [0]
                prev = None
                for r, (qt, hp) in enumerate(ROUNDS):
                    e2s = emit_round(qt, hp, prev, BG[r])
                    prev = (qt, hp, e2s)

                # ---- tail: PV of last round interleaved with oproj(2)
                # (its ot3 half is written by pv_norm(2,1) at the end of
                # round 7), then oproj(3) with ACT/DVE-alternating copies.
                pqt, php, pe2s = prev
                pva = pspv.tile([DK + 1, 512], fp32, tag="pv")
                pvb = pspv.tile([DK + 1, 512], fp32, tag="pv")
                for kt in range(NKT):
                    pv_kt(php, pe2s, kt, pva, pvb)
                    if kt % 2 == 1:
                        oproj_unit(2, kt // 2, on_act=(kt % 4 == 1))
                pv_norm(pqt, php, pva, pvb)
                for jt in range(NDT):
                    oproj_unit(3, jt, on_act=(jt % 2 == 0))

    nc.compile()
    return nc


def kernel(q, k, v, Wq, bq, Wk, bk, Wv, bv, Wo, bo, _trace=False):
    from concourse import bass_utils

    if "nc" not in _cache:
        _cache["nc"] = _build()
    nc = _cache["nc"]

    q = np.asarray(q, np.float32)
    k = np.asarray(k, np.float32)
    v = np.asarray(v, np.float32)
    Wq = np.asarray(Wq, np.float32)
    Wk = np.asarray(Wk, np.float32)
    Wv = np.asarray(Wv, np.float32)
    Wo = np.asarray(Wo, np.float32)
    bq = np.asarray(bq, np.float32)
    bk = np.asarray(bk, np.float32)
    bv = np.asarray(bv, np.float32)
    bo = np.asarray(bo, np.float32)

    f16 = np.float16

    # host-side pre-arrangement: all DMA rows contiguous per partition
    def arr_x_q(xT):    # [D,S] -> [128, NQT, NDT, 512]
        return np.ascontiguousarray(
            xT.reshape(NDT, 128, NQT, 512).transpose(1, 2, 0, 3)).astype(f16)

    def arr_x_v(xT):    # [D,S] -> [128, NKT, NDT, 128]
        return np.ascontiguousarray(
            xT.reshape(NDT, 128, NKT, 128).transpose(1, 2, 0, 3)).astype(f16)

    def arr_w(WslT):    # [D, FEAT] -> [128, NDT, FEAT]
        return np.ascontiguousarray(
            WslT.reshape(NDT, 128, FEAT).transpose(1, 0, 2)).astype(f16)

    xT = {}
    for b in range(B):
        xT[("q", b)] = arr_x_q(q[b].T)
        xT[("k", b)] = arr_x_q(k[b].T)
        xT[("v", b)] = arr_x_v(v[b].T)
    wT = {}
    for g in range(HG):
        sl = slice(g * FEAT, (g + 1) * FEAT)
        wT[("q", g)] = arr_w(Wq[sl, :].T)
        wT[("k", g)] = arr_w(Wk[sl, :].T)
        wT[("v", g)] = arr_w(Wv[sl, :].T)
        wT[("o", g)] = np.ascontiguousarray(
            Wo[:, sl].T.reshape(2, 128, D).transpose(1, 0, 2)).astype(f16)

    in_maps = []
    for c in range(N_CORES):
        b, g = divmod(c, HG)
        sl = slice(g * FEAT, (g + 1) * FEAT)
        in_maps.append({
            "xq_d": xT[("q", b)], "xk_d": xT[("k", b)], "xv_d": xT[("v", b)],
            "wq_d": wT[("q", g)], "wk_d": wT[("k", g)], "wv_d": wT[("v", g)],
            "wo_d": wT[("o", g)],
            "bq_d": np.ascontiguousarray(
                bq[sl].reshape(2, 128).T.reshape(128, 2, 1)),
            "bk_d": np.ascontiguousarray(
                bk[sl].reshape(2, 128).T.reshape(128, 2, 1)),
        })

    kwargs = {}
    if _trace:
        _install_profile_shim()
        kwargs = dict(trace=True, trace_cores=list(range(N_CORES)))
    res = bass_utils.run_bass_kernel_spmd(
        nc, in_maps, core_ids=list(range(N_CORES)), **kwargs)
    _cache["last_results"] = res

    final_bias = (Wo @ bv + bo).astype(np.float32)  # attn rows sum to 1
    out = np.empty((B, S, D), np.float32)
    for b in range(B):
        acc = res.results[b * HG]["partialT"].astype(np.float32)
        for g in range(1, HG):
            acc += res.results[b * HG + g]["partialT"].astype(np.float32)
        out[b] = acc.T + final_bias
    return out


def _install_profile_shim():
    """Provide antenv.axon_hooks so trace=True works under axon."""
    import sys
    import types

    import antenv

    if "antenv.axon_hooks" in sys.modules:
        return
    mod = types.ModuleType("antenv.axon_hooks")
    mod._hook = None
    mod.set_axon_ntff_profile_hook = lambda h: setattr(mod, "_hook", h)
    mod.get_axon_ntff_profile_hook = lambda: mod._hook
    sys.modules["antenv.axon_hooks"] = mod
    antenv.axon_hooks = mod
    try:
        from trn_agent_boot.trn_boot import _ntff_profile_via_ctypes
        mod.set_axon_ntff_profile_hook(
            _ntff_profile_via_ctypes("/opt/axon/libaxon_pjrt.so"))
    except Exception:
        pass
